# revision 1
# baseline (speedup 1.0000x reference)
"""DyConv2d (dynamic convolution with SE attention) on 8 TRN2 NeuronCores.

Reference computation (per image):
    attn = softmax(MLP(global_avg_pool(x)) / T)            # [K=4]
    y    = conv3x3(x, W) + bias                            # W: [K*128, 128, 3, 3]
    out  = sum_k attn[k] * y[k]                            # [128, 64, 64]

Key algebraic rewrite: conv is linear in the weights, so
    out = conv3x3(x, sum_k attn[k] * W_k) + sum_k attn[k] * bias_k
which cuts the conv FLOPs by 4x (one 128->128 conv per image instead of
128->512).

Sharding: data-parallel over batch, 2 images per core. The replicated
weights are laid out host-side in the transposed [k, ci, tap, co] order the
TensorE needs (lhsT), so no on-device transposes are required and the
per-tap-group weight DMAs pipeline with the attention computation.

Per-core pipeline (engine assignment keeps the PE the bottleneck):
  1. x DMA (sync HWDGE, 2 halves) -> DVE re-rounds to float32r (the PE's
     full-rate fp32 mode, ~1.5e-4 rel err) into a flat-padded layout and
     emits the SE global sum via accum_out.
  2. Wt DMA (scalar HWDGE queue) in tap-group-major order so the first
     combine group unblocks after ~1/3 of the weight bytes.
  3. Per-image SE MLP on PE (tiny, exact f32); softmax on ACT/DVE with two
     tiny DRAM bounces for the [4,1]->[1,4] transpose and the 128-partition
     attn broadcast (DRAM APs allow partition-stride-0).
  4. Per-image weight combine over k on DVE in 3 groups of 3 taps (fused
     scalar_tensor_tensor chain, final write rounds to f32r), so the conv
     starts right after group 0.
  5. Conv: flat-padded layout with row pitch 65 -> each row's right pad
     aliases the next row's left pad (zero), so every 3x3 tap is one fully
     contiguous fp32r matmul at flat offset dy*65+dx. Tap-major over groups
     of 2-3 row-blocks (PSUM banks), 9 accumulating matmuls per bank,
     N = 7*65+1 = 456 (fp32r requires even N <= 512).
  6. Eviction adds the attn-combined bias on ACT (Identity + bias AP) and
     DMAs out on alternating HWDGE queues.
"""

import sys

sys.path.insert(0, "/opt/trn_rl_repo")

import numpy as np

from concourse import bacc, mybir
import concourse.tile as tile
from concourse.bass_utils import run_bass_kernel_spmd
from concourse.tile_rust import add_dep_helper

B_TOTAL = 16
N_CORES = 8
B = B_TOTAL // N_CORES  # images per core
CI = 128
CO = 128
K = 4
H = W = 64
TEMP = 30.0
F32 = mybir.dt.float32
F32R = mybir.dt.float32r

# raster order: tap i = (i//3, i%3), matching the weff group layout
TAPS = [(i // 3, i % 3) for i in range(9)]
XPL = 65 * 66 + 4  # padded-x flat length (extra zeros absorb window overrun)
BLOCKS = [(h0, 7) for h0 in range(0, 63, 7)] + [(63, 1)]
BGROUPS = [BLOCKS[0:3], BLOCKS[3:6], BLOCKS[6:8], BLOCKS[8:10]]

_NC_CACHE = {}


def build_nc(reps=1):
    nc = bacc.Bacc("TRN2", target_bir_lowering=False)

    x_d = nc.dram_tensor("x2", [B, CI, H, W], F32, kind="ExternalInput")
    wt_d = nc.dram_tensor("weight_t", [K, CI, 9, CO], F32, kind="ExternalInput")
    bc_d = nc.dram_tensor("bias_cos", [CO, K], F32, kind="ExternalInput")
    w1t_d = nc.dram_tensor("se_w1t", [CI, 33], F32, kind="ExternalInput")
    w2t_d = nc.dram_tensor("se_w2t", [33, K], F32, kind="ExternalInput")
    b2_d = nc.dram_tensor("se_b2", [K], F32, kind="ExternalInput")
    y_d = nc.dram_tensor("y2", [B, CO, H, W], F32, kind="ExternalOutput")

    with tile.TileContext(nc) as tc:
        with (
            tc.tile_pool(name="consts", bufs=1) as consts,
            tc.tile_pool(name="ximg", bufs=2) as ximg,
            tc.tile_pool(name="weff", bufs=6) as weffp,
            tc.tile_pool(name="cmb", bufs=2) as cmbp,
            tc.tile_pool(name="sesb", bufs=2) as sesb,
            tc.tile_pool(name="ev", bufs=4) as evp,
            tc.tile_pool(name="cv", bufs=6, space="PSUM") as cvp,
            tc.tile_pool(name="tp", bufs=2, space="PSUM") as tpp,
        ):
            for _ in range(reps):
                build_body(nc, tc, consts, ximg, weffp, cmbp, sesb, evp, cvp,
                           tpp, x_d, wt_d, bc_d, w1t_d, w2t_d, b2_d, y_d)

    nc.compile()
    return nc


def build_body(nc, tc, consts, ximg, weffp, cmbp, sesb, evp, cvp, tpp,
               x_d, wt_d, bc_d, w1t_d, w2t_d, b2_d, y_d):
    pooled = consts.tile([128, B], F32, tag="pooled")
    pool_parts = consts.tile([128, B, 2], F32, tag="pool_parts")
    lg_dram = nc.dram_tensor("lg_bounce", [B, K], F32)
    attn_dram = nc.dram_tensor("attn_bounce", [B, K], F32)
    x_sb = [None, None]
    x_r = [None, None]

    def load_x(b):
        t = ximg.tile([128, H, W], F32, tag=f"x_sb{b}", name=f"x_sb{b}")
        nc.sync.dma_start(out=t[:, 0:32, :], in_=x_d[b, :, 0:32, :])
        nc.sync.dma_start(out=t[:, 32:64, :], in_=x_d[b, :, 32:64, :])
        x_sb[b] = t

    def round_image(b):
        """f32r-round x into the flat-padded layout; accumulate the SE sums."""
        xr = ximg.tile([128, XPL], F32R, tag=f"x_r{b}", name=f"x_r{b}")
        xr_rows = xr[:, 0:65 * 66].rearrange("p (r c) -> p r c", c=65)
        x_flat = x_sb[b].rearrange("p a b -> p (a b)")
        # zero the pad cells; memset can't produce float32r, so use in*0 ops
        for pad_out, pad_in in [
            (xr[:, 0:66], x_flat[:, 0:66]),            # top pad row
            (xr_rows[:, 2:65, 0], x_flat[:, 0:63]),    # left pads
            (xr[:, 65 * 65:XPL], x_flat[:, 0:69]),     # bottom pad row
        ]:
            nc.vector.tensor_scalar(
                out=pad_out, in0=pad_in, scalar1=0.0, scalar2=None,
                op0=mybir.AluOpType.mult,
            )
        for hh in (0, 1):  # round each 32-row half as its DMA lands
            nc.vector.tensor_scalar(
                out=xr_rows[:, 1 + 32 * hh:1 + 32 * (hh + 1), 1:65],
                in0=x_sb[b][:, 32 * hh:32 * (hh + 1), :],
                scalar1=1.0, scalar2=0.0,
                op0=mybir.AluOpType.mult, op1=mybir.AluOpType.add,
                accum_out=pool_parts[:, b, hh:hh + 1],
            )
        nc.vector.tensor_add(pooled[:, b:b + 1], pool_parts[:, b, 0:1],
                             pool_parts[:, b, 1:2])
        x_r[b] = xr

    # ---- weights (already [k, ci, tap, co] from the host), group-major ----
    wt = [consts.tile([128, 9, CO], F32, tag=f"wt{k}", name=f"wt{k}")
          for k in range(K)]

    def load_w_group(g):
        # one contiguous DMA per k (4.6KB/partition runs, max DMA efficiency)
        if g == 0:
            for k in range(K):
                nc.scalar.dma_start(out=wt[k], in_=wt_d[k])

    # tiny SE params first: a few KB that gate the whole attention chain
    w1t_sb = consts.tile([CI, 33], F32, tag="w1t_sb")
    nc.scalar.dma_start(out=w1t_sb, in_=w1t_d[:, :])
    w2t_sb = consts.tile([33, K], F32, tag="w2t_sb")
    nc.scalar.dma_start(out=w2t_sb, in_=w2t_d[:, :])
    b2_sb = consts.tile([K, 1], F32, tag="b2_sb")
    nc.scalar.dma_start(out=b2_sb, in_=b2_d[:].rearrange("(a b) -> a b", b=1))
    bias_cos = consts.tile([CO, K], F32, tag="bias_cos")
    nc.scalar.dma_start(out=bias_cos, in_=bc_d[:, :])
    load_x(0)
    load_w_group(0)
    round_image(0)

    cb_all = consts.tile([128, B], F32, tag="cb_all")

    def se_attn(b):
        """SE MLP + softmax for one image -> attn_bc [128, K]; cb into cb_all."""
        ps_h = tpp.tile([128, 512], F32, tag="tp", name="ps_h")[0:33, 0:1]
        nc.tensor.matmul(ps_h, w1t_sb, pooled[:, b:b + 1], start=True, stop=True)
        h_sb = sesb.tile([33, 1], F32, tag="h_sb")
        nc.scalar.activation(out=h_sb, in_=ps_h,
                             func=mybir.ActivationFunctionType.Relu,
                             scale=1.0 / (H * W))
        ps_lg = tpp.tile([128, 512], F32, tag="tp", name="ps_lg")[0:K, 0:1]
        nc.tensor.matmul(ps_lg, w2t_sb, h_sb, start=True, stop=True)
        lg_sb = sesb.tile([K, 1], F32, tag="lg_sb")
        nc.scalar.activation(out=lg_sb, in_=ps_lg,
                             func=mybir.ActivationFunctionType.Identity,
                             bias=b2_sb[:, 0:1], scale=1.0)
        # [4,1] -> [1,4] via a tiny DRAM bounce (DRAM APs are layout-free)
        nc.sync.dma_start(out=lg_dram[b], in_=lg_sb)
        lgt = sesb.tile([1, K], F32, tag="lgt")
        nc.sync.dma_start(out=lgt, in_=lg_dram[b].rearrange("(a k) -> a k", a=1))
        e_sb = sesb.tile([1, K], F32, tag="e_sb")
        nc.scalar.activation(out=e_sb, in_=lgt,
                             func=mybir.ActivationFunctionType.Exp,
                             scale=1.0 / TEMP)
        s_sb = sesb.tile([1, 1], F32, tag="s_sb")
        nc.vector.reduce_sum(out=s_sb, in_=e_sb, axis=mybir.AxisListType.X)
        r_sb = sesb.tile([1, 1], F32, tag="r_sb")
        nc.vector.reciprocal(out=r_sb, in_=s_sb)
        attn = sesb.tile([1, K], F32, tag="attn")
        nc.vector.tensor_scalar_mul(attn, e_sb, r_sb[:, 0:1])
        # broadcast to 128 partitions via DRAM bounce (partition stride 0)
        nc.sync.dma_start(out=attn_dram[b], in_=attn)
        attn_bc = sesb.tile([128, K], F32, tag="attn_bc")
        nc.sync.dma_start(out=attn_bc, in_=attn_dram[b].partition_broadcast(128))
        # combined bias cb = sum_k attn[k] * bias[k]
        tmp = sesb.tile([128, K], F32, tag="cbtmp")
        nc.vector.tensor_mul(tmp, bias_cos, attn_bc)
        nc.vector.reduce_sum(out=cb_all[:, b:b + 1], in_=tmp,
                             axis=mybir.AxisListType.X)
        return attn_bc

    def combine_group(attn_bc, g):
        """weff_g [128, 3, CO] (f32r) = sum_k attn[k] * wt[k][:, 3g:3g+3, :]"""
        sl = slice(3 * g, 3 * g + 3)
        t0 = cmbp.tile([128, 3, CO], F32, tag="cmb_t")
        nc.vector.tensor_scalar(
            out=t0, in0=wt[0][:, sl, :], scalar1=attn_bc[:, 0:1],
            scalar2=None, op0=mybir.AluOpType.mult)
        t1 = cmbp.tile([128, 3, CO], F32, tag="cmb_t")
        nc.vector.scalar_tensor_tensor(
            out=t1, in0=wt[1][:, sl, :], scalar=attn_bc[:, 1:2], in1=t0,
            op0=mybir.AluOpType.mult, op1=mybir.AluOpType.add)
        t2 = cmbp.tile([128, 3, CO], F32, tag="cmb_t")
        nc.vector.scalar_tensor_tensor(
            out=t2, in0=wt[2][:, sl, :], scalar=attn_bc[:, 2:3], in1=t1,
            op0=mybir.AluOpType.mult, op1=mybir.AluOpType.add)
        wg = weffp.tile([128, 3, CO], F32R, tag="weff")
        last = nc.vector.scalar_tensor_tensor(
            out=wg, in0=wt[3][:, sl, :], scalar=attn_bc[:, 3:4], in1=t2,
            op0=mybir.AluOpType.mult, op1=mybir.AluOpType.add)
        return wg, last

    def conv_image(b, weff_groups):
        xr = x_r[b]
        for gi, grp in enumerate(BGROUPS):
            pss = [cvp.tile([128, 512], F32, tag="cv", name=f"cv{j}")
                   for j in range(len(grp))]
            grows = sum(nr for _, nr in grp)
            gh0 = grp[0][0]
            out_sb = evp.tile([128, 21, W], F32, tag="ev", name="ev_g")
            for i, (ky, kx) in enumerate(TAPS):
                lhsT = weff_groups[i // 3][:, i % 3, :]
                off = (ky - 1) * 65 + (kx - 1)
                for j, (ps, (h0, nr)) in enumerate(zip(pss, grp)):
                    n = nr * 65 + 1  # +1 keeps N even (fp32r requires it)
                    obase = (h0 + 1) * 65 + 1
                    nc.tensor.matmul(
                        ps[:, 0:n], lhsT, xr[:, obase + off:obase + off + n],
                        start=(i == 0), stop=(i == 8),
                    )
                    if i == 8:
                        # evict right away (bias add) into the group staging
                        # tile so the PSUM bank frees while the PE finishes
                        # the remaining stop-tap matmuls
                        r0 = h0 - gh0
                        ps_rows = ps[:, 0:455].rearrange("p (r c) -> p r c",
                                                         c=65)
                        nc.scalar.activation(
                            out=out_sb[:, r0:r0 + nr, :],
                            in_=ps_rows[:, 0:nr, 0:64],
                            func=mybir.ActivationFunctionType.Identity,
                            bias=cb_all[:, b:b + 1], scale=1.0)
            # one large contiguous DMA per block-group (fewer descriptors)
            dma_eng = nc.sync if gi % 2 == 0 else nc.scalar
            dma_eng.dma_start(out=y_d[b, :, gh0:gh0 + grows, :],
                              in_=out_sb[:, 0:grows, :])

    # image 0: SE -> combine groups -> conv (taps of group g follow combine g)
    attn0 = se_attn(0)
    w0 = [combine_group(attn0, g)[0] for g in range(3)]
    load_x(1)
    conv_image(0, w0)
    # image 1 prep (placed after image-0 combine so it can't delay it on DVE)
    round_image(1)
    attn1 = se_attn(1)
    w1 = [combine_group(attn1, g)[0] for g in range(3)]
    conv_image(1, w1)


def get_nc():
    if "nc" not in _NC_CACHE:
        _NC_CACHE["nc"] = build_nc()
    return _NC_CACHE["nc"]


def shard_inputs(x, weight, bias, se_w1, se_w2, se_b2):
    # host-side layout prep of the replicated (batch-independent) params:
    # weight -> [k, ci, tap, co] (the lhsT layout the TensorE consumes)
    w4 = np.ascontiguousarray(weight, np.float32).reshape(K, CO, CI, 3, 3)
    weight_t = np.ascontiguousarray(w4.transpose(0, 2, 3, 4, 1)
                                    .reshape(K, CI, 9, CO))
    common = dict(
        weight_t=weight_t,
        bias_cos=np.ascontiguousarray(
            np.asarray(bias, np.float32).reshape(K, CO).T),
        se_w1t=np.ascontiguousarray(np.asarray(se_w1, np.float32).T),
        se_w2t=np.ascontiguousarray(np.asarray(se_w2, np.float32).T),
        se_b2=np.ascontiguousarray(se_b2, np.float32),
    )
    return [
        dict(x2=np.ascontiguousarray(x[c * B:(c + 1) * B], np.float32), **common)
        for c in range(N_CORES)
    ]


def kernel(x, weight, bias, se_w1, se_w2, se_b2):
    nc = get_nc()
    in_maps = shard_inputs(x, weight, bias, se_w1, se_w2, se_b2)
    res = run_bass_kernel_spmd(nc, in_maps, core_ids=list(range(N_CORES)))
    return np.concatenate([r["y2"] for r in res.results], axis=0)



# revision 16
# speedup vs baseline: 1.6991x; 1.6991x over previous
"""DyConv2d (dynamic convolution with SE attention) on 8 TRN2 NeuronCores.

Reference computation (per image):
    attn = softmax(MLP(global_avg_pool(x)) / T)            # [K=4]
    y    = conv3x3(x, W) + bias                            # W: [K*128, 128, 3, 3]
    out  = sum_k attn[k] * y[k]                            # [128, 64, 64]

Key algebraic rewrite: conv is linear in the weights, so
    out = conv3x3(x, sum_k attn[k] * W_k) + sum_k attn[k] * bias_k
which cuts the conv FLOPs by 4x (one 128->128 conv per image instead of
128->512).

Sharding: data-parallel over batch, 2 images per core.

This revision keeps the whole SE-attention chain on-chip (the previous
version bounced tiny tensors through DRAM, and those DMAs queued behind
megabytes of bulk input traffic):
  * logits are produced directly in [1, K] row layout by swapping the
    matmul operand roles (lhsT = hidden column, rhs = W2^T), absorbing the
    [K,1] -> [1,K] transpose into the existing matmul,
  * the 128-partition broadcast of the attention row is a matmul against a
    ones [1,128] stationary vector,
  * softmax normalisation happens after the broadcast with three tiny DVE
    ops, so the combine can start immediately.

The conv runs in bf16 (x and the combined weights are rounded on DVE; PSUM
accumulates in fp32), which is full-rate on the PE for any tile size and
halves the weight DMA bytes. Raw weights are shipped bf16 in tap-major
layout [ci, tap, k, co] so each combine group (tap groups 1/2/3/3) is one
contiguous DMA that lands just-in-time.

Per-core pipeline:
  1. x image 0 DMA in 4 quarters (sync queue); DVE rounds each quarter to
     bf16 into a flat-padded layout (row pitch 65 so every 3x3 tap is one
     contiguous matmul) and emits the SE pooling sums via accum_out.
  2. bf16 weights + tiny SE params DMA on the scalar queue.
  3. SE: matmul -> relu -> matmul (transposed logits) -> exp -> broadcast
     matmul -> normalize on DVE -> combined bias on DVE.
  4. Weight combine over k on DVE in tap groups (1,2,3,3), fp32 chain with
     a final bf16 round, so the conv starts right after group 0.
  5. Conv: block-groups of 7-row PSUM banks, 9 accumulating bf16 matmuls
     per bank (N=454), evicted per block on ACT (adds the attn-combined
     bias) and DMA'd out per block on the Pool SWDGE queue.
  6. Image 1's SE matmuls are spliced between image-0 conv block-groups so
     they never stall the PE for long.
"""

import sys

sys.path.insert(0, "/opt/trn_rl_repo")

import numpy as np

from concourse import bacc, mybir
import concourse.tile as tile
from concourse.bass_utils import run_bass_kernel_spmd

B_TOTAL = 16
N_CORES = 8
B = B_TOTAL // N_CORES  # images per core
CI = 128
CO = 128
K = 4
H = W = 64
TEMP = 30.0
F32 = mybir.dt.float32
BF16 = mybir.dt.bfloat16

# raster order: tap i = (i//3, i%3)
TAPS = [(i // 3, i % 3) for i in range(9)]
XPL = 65 * 66 + 4  # padded-x flat length (extra zeros absorb window overrun)
BLOCKS = [(h0, 7) for h0 in range(0, 63, 7)] + [(63, 1)]
BGROUPS = [BLOCKS[0:3], BLOCKS[3:6], BLOCKS[6:8], BLOCKS[8:10]]
# tap groups for the weight combine: sizes 1,2,3,3 so group 0 is ready asap
TAPG = [(0, 1), (1, 2), (3, 3), (6, 3)]
TAPG_OF = [0, 1, 1, 2, 2, 2, 3, 3, 3]

_NC_CACHE = {}


def build_nc():
    nc = bacc.Bacc("TRN2", target_bir_lowering=False)

    x_d = nc.dram_tensor("x2", [B, CI, H, W], BF16, kind="ExternalInput")
    wt_d = nc.dram_tensor("weight_t", [CI, 9, K, CO], BF16, kind="ExternalInput")
    bc_d = nc.dram_tensor("bias_cos", [CO, K], F32, kind="ExternalInput")
    w1t_d = nc.dram_tensor("se_w1t", [CI, 33], F32, kind="ExternalInput")
    w2b_d = nc.dram_tensor("se_w2b", [34, K], F32, kind="ExternalInput")
    y_d = nc.dram_tensor("y2", [B, CO, H, W], BF16, kind="ExternalOutput")

    with tile.TileContext(nc) as tc:
        with (
            tc.tile_pool(name="consts", bufs=1) as consts,
            tc.tile_pool(name="ximg", bufs=2) as ximg,
            tc.tile_pool(name="weff", bufs=8) as weffp,
            tc.tile_pool(name="cmb", bufs=6) as cmbp,
            tc.tile_pool(name="sesb", bufs=2) as sesb,
            tc.tile_pool(name="ev", bufs=4) as evp,
            tc.tile_pool(name="cv", bufs=6, space="PSUM") as cvp,
            tc.tile_pool(name="tp", bufs=2, space="PSUM") as tpp,
        ):
            build_body(nc, tc, consts, ximg, weffp, cmbp, sesb, evp, cvp, tpp,
                       x_d, wt_d, bc_d, w1t_d, w2b_d, y_d)

    nc.compile()
    return nc


def build_body(nc, tc, consts, ximg, weffp, cmbp, sesb, evp, cvp, tpp,
               x_d, wt_d, bc_d, w1t_d, w2b_d, y_d):
    pool_parts = consts.tile([128, B, 8], F32, tag="pool_parts")
    x_sb = [None, None]
    x_r = [None, None]

    # ---- tiny SE params on the Pool SWDGE queue (keeps HWDGE free) -------
    w1t_sb = consts.tile([CI, 33], F32, tag="w1t_sb")
    nc.gpsimd.dma_start(out=w1t_sb, in_=w1t_d[:, :])
    w2b_sb = consts.tile([34, K], F32, tag="w2b_sb")
    nc.gpsimd.dma_start(out=w2b_sb, in_=w2b_d[:, :])
    bias_cos = consts.tile([CO, K], F32, tag="bias_cos")
    nc.gpsimd.dma_start(out=bias_cos, in_=bc_d[:, :])

    # x chunk row splits: image 0 ends with small chunks so the last
    # bf16-round (the pooled critical path) is short
    XCHUNKS0 = [16, 16, 16, 12, 4]
    XCHUNKS1 = [32, 32]

    def load_x(b, chunks):
        t = ximg.tile([128, H, W], BF16, tag=f"x_sb{b}", name=f"x_sb{b}")
        r0 = 0
        for rows in chunks:
            nc.sync.dma_start(out=t[:, r0:r0 + rows, :],
                              in_=x_d[b, :, r0:r0 + rows, :])
            r0 += rows
        x_sb[b] = t

    load_x(0, XCHUNKS0)

    # raw bf16 weights, tap-major [ci, tap, k, co]; one DMA per tap group,
    # on the same sync queue as x so the ordering x0 -> w -> x1 is exact
    wt_all = consts.tile([128, 9, K, CO], BF16, tag="wt_all")
    for t0, nt in TAPG:
        nc.sync.dma_start(out=wt_all[:, t0:t0 + nt, :, :],
                          in_=wt_d[:, t0:t0 + nt, :, :])

    # constants for the on-chip SE chain (Pool engine memsets, no deps)
    ones_sb = consts.tile([1, 128], F32, tag="ones_sb")
    nc.gpsimd.memset(ones_sb, 1.0)
    # dummy no-dep activation so the ACT function-table load (1.3us) runs
    # at t=0 instead of blocking the first relu on the critical path
    warm_sb = consts.tile([1, 1], F32, tag="warm_sb")
    nc.scalar.activation(out=warm_sb, in_=ones_sb[:, 0:1],
                         func=mybir.ActivationFunctionType.Exp)
    h34 = consts.tile([34, 1], F32, tag="h34")
    nc.gpsimd.memset(h34, 1.0)  # row 33 stays 1.0; relu overwrites rows 0-32

    def pad_xr(b):
        """Allocate the flat-padded bf16 image tile and zero its pads."""
        xr = ximg.tile([128, XPL], BF16, tag=f"x_r{b}", name=f"x_r{b}")
        xr_rows = xr[:, 0:65 * 66].rearrange("p (r c) -> p r c", c=65)
        nc.gpsimd.memset(xr[:, 0:66], 0.0)            # top pad row (+row1 col0)
        nc.gpsimd.memset(xr_rows[:, 2:65, 0], 0.0)    # left pads
        nc.gpsimd.memset(xr[:, 65 * 65:XPL], 0.0)     # bottom pad row + extra
        x_r[b] = xr

    def round_image(b, chunks):
        """copy bf16 x into the flat-padded layout (4x-packed DVE mode);
        per-chunk SE sums go to pool_parts via accum_out (summed by
        accumulating matmuls in se_mm1)."""
        xr = x_r[b]
        xr_rows = xr[:, 0:65 * 66].rearrange("p (r c) -> p r c", c=65)
        r0 = 0
        for q, rows in enumerate(chunks):
            nc.vector.tensor_scalar(
                out=xr_rows[:, 1 + r0:1 + r0 + rows, 1:65],
                in0=x_sb[b][:, r0:r0 + rows, :],
                scalar1=1.0, scalar2=0.0,
                op0=mybir.AluOpType.mult, op1=mybir.AluOpType.add,
                accum_out=pool_parts[:, b, q:q + 1],
            )
            r0 += rows

    pad_xr(0)
    pad_xr(1)
    round_image(0, XCHUNKS0)

    cb_all = consts.tile([128, B], F32, tag="cb_all")
    r_all = consts.tile([128, B], F32, tag="r_all")

    # --- SE chain, split into PE-step functions so image 1's matmuls can
    # be spliced between image-0 conv block-groups without stalling the PE.
    def se_mm1(b, nchunks):
        # h_pre = W1 @ pooled, accumulated per x-chunk in PSUM: chunk q's
        # partial matmul runs as soon as its round lands, so only the last
        # (smallest) chunk is on the critical path
        ps_h = tpp.tile([128, 512], F32, tag="tp", name=f"ps_h{b}")[0:33, 0:1]
        for q in range(nchunks):
            nc.tensor.matmul(ps_h, w1t_sb, pool_parts[:, b, q:q + 1],
                             start=(q == 0), stop=(q == nchunks - 1))
        nc.scalar.activation(out=h34[0:33, :], in_=ps_h,
                             func=mybir.ActivationFunctionType.Relu,
                             scale=1.0 / (H * W))

    def se_mm2(b):
        # logits directly in [1, K] row layout: lhsT = h column, rhs = W2^T
        # (bias folded in via h34[33] = 1, w2b row 33 = b2); softmax done on
        # the tiny row so the broadcast ships already-normalized attention
        ps_lgt = tpp.tile([128, 512], F32, tag="tp", name=f"ps_lgt{b}")[0:1, 0:K]
        nc.tensor.matmul(ps_lgt, h34, w2b_sb, start=True, stop=True)
        e_row = sesb.tile([1, K], F32, tag="e_row", name=f"e_row{b}")
        nc.scalar.activation(out=e_row, in_=ps_lgt,
                             func=mybir.ActivationFunctionType.Exp,
                             scale=1.0 / TEMP)
        return e_row

    def se_bcast(b, e_row):
        # broadcast the UNnormalized exp row to 128 partitions via a
        # ones-vector matmul, then one ACT copy into SBUF (PSUM-sourced
        # scalars cost every DVE combine op an extra ~130ns of init);
        # the softmax 1/sum is folded into the eviction's scale AP
        ps_e = tpp.tile([128, 512], F32, tag="tp", name=f"ps_e{b}")[:, 0:K]
        nc.tensor.matmul(ps_e, ones_sb, e_row, start=True, stop=True)
        e_bc = sesb.tile([128, K], F32, tag="e_bc", name=f"e_bc{b}")
        nc.vector.tensor_scalar(out=e_bc, in0=ps_e, scalar1=1.0, scalar2=None,
                                op0=mybir.AluOpType.mult)
        return e_bc

    def se_norm(b, ps_e):
        # r = 1/sum_k e_k, per partition (off the conv critical path)
        s_bc = sesb.tile([128, 1], F32, tag="s_bc")
        nc.vector.reduce_sum(out=s_bc, in_=ps_e, axis=mybir.AxisListType.X)
        nc.vector.reciprocal(out=r_all[:, b:b + 1], in_=s_bc)

    def comb_bias(b, ps_e):
        # cbr = r * sum_k e_k * bias[k]  (the normalized combined bias)
        tmp = sesb.tile([128, K], F32, tag="cbtmp")
        nc.vector.tensor_mul(tmp, bias_cos, ps_e)
        cbe = sesb.tile([128, 1], F32, tag="cbe")
        nc.vector.reduce_sum(out=cbe, in_=tmp, axis=mybir.AxisListType.X)
        nc.vector.tensor_mul(cb_all[:, b:b + 1], cbe, r_all[:, b:b + 1])

    def combine_group(b, attn_bc, g):
        """weff_g [128, nt, CO] (bf16) = sum_k attn[k] * wt[:, taps_g, k, :].

        All-bf16 tensor_scalar / tensor_tensor run in the DVE's packed 4x
        mode; scalar_tensor_tensor does not, so the sum is a scale/add tree
        rather than a multiply-accumulate chain."""
        t0, nt = TAPG[g]
        sl = slice(t0, t0 + nt)
        tk = []
        for k in range(K):
            t = cmbp.tile([128, 3, CO], BF16, tag="cmb_t",
                          name=f"cmb_t{k}")[:, 0:nt, :]
            nc.vector.tensor_scalar(
                out=t, in0=wt_all[:, sl, k, :], scalar1=attn_bc[:, k:k + 1],
                scalar2=None, op0=mybir.AluOpType.mult)
            tk.append(t)
        t01 = cmbp.tile([128, 3, CO], BF16, tag="cmb_t",
                        name="cmb_t01")[:, 0:nt, :]
        nc.vector.tensor_add(t01, tk[0], tk[1])
        t23 = cmbp.tile([128, 3, CO], BF16, tag="cmb_t",
                        name="cmb_t23")[:, 0:nt, :]
        nc.vector.tensor_add(t23, tk[2], tk[3])
        wg = weffp.tile([128, nt, CO], BF16, tag=f"weff{b}g{g}",
                        name=f"weff{b}g{g}")
        nc.vector.tensor_add(wg, t01, t23)
        return wg

    def conv_bgroup(b, weff, grp, dma_eng=None):
        """One block-group: 9 accumulating taps into len(grp) PSUM banks,
        per-block ACT eviction (bias add) into a group staging tile, one
        Pool-queue DMA per group (or per block when split_dma, to shrink
        the final-DMA tail)."""
        xr = x_r[b]
        pss = [cvp.tile([128, 512], F32, tag="cv", name=f"cv{b}_{grp[0][0]}_{j}")
               for j in range(len(grp))]
        grows = sum(nr for _, nr in grp)
        gh0 = grp[0][0]
        out_sb = evp.tile([128, 21, W], BF16, tag="ev", name=f"ev{b}_{gh0}")
        for i, (ky, kx) in enumerate(TAPS):
            g = TAPG_OF[i]
            lhsT = weff[g][:, i - TAPG[g][0], :]
            off = (ky - 1) * 65 + (kx - 1)
            for j, (ps, (h0, nr)) in enumerate(zip(pss, grp)):
                n = nr * 65 - 1 if nr > 1 else 64
                obase = (h0 + 1) * 65 + 1
                nc.tensor.matmul(
                    ps[:, 0:n], lhsT, xr[:, obase + off:obase + off + n],
                    start=(i == 0), stop=(i == 8),
                )
                if i == 8:
                    r0 = h0 - gh0
                    ps_rows = ps[:, 0:nr * 65].rearrange("p (r c) -> p r c",
                                                         c=65)
                    nc.scalar.activation(
                        out=out_sb[:, r0:r0 + nr, :],
                        in_=ps_rows[:, 0:nr, 0:64],
                        func=mybir.ActivationFunctionType.Identity,
                        bias=cb_all[:, b:b + 1],
                        scale=r_all[:, b:b + 1])
        (dma_eng or nc.gpsimd).dma_start(out=y_d[b, :, gh0:gh0 + grows, :],
                                         in_=out_sb[:, 0:grows, :])

    # ---------------- image 0 prologue ----------------
    load_x(1, XCHUNKS1)  # x image 1 queued behind the weights (sync queue)
    se_mm1(0, len(XCHUNKS0))
    e0 = se_mm2(0)
    p0 = se_bcast(0, e0)
    wg00 = combine_group(0, p0, 0)
    se_norm(0, p0)
    w0 = [wg00] + [combine_group(0, p0, g) for g in range(1, 4)]
    comb_bias(0, p0)

    # conv image 0, with image-1 SE matmuls spliced between block-groups
    conv_bgroup(0, w0, BGROUPS[0])
    round_image(1, XCHUNKS1)
    conv_bgroup(0, w0, BGROUPS[1])
    se_mm1(1, len(XCHUNKS1))  # PE: runs right after group B's last matmul
    e1 = se_mm2(1)   # PE: short wait on ACT relu only
    conv_bgroup(0, w0, BGROUPS[2])
    p1 = se_bcast(1, e1)  # PE broadcast after group C
    wg10 = combine_group(1, p1, 0)
    se_norm(1, p1)
    w1 = [wg10] + [combine_group(1, p1, g) for g in range(1, 4)]
    comb_bias(1, p1)
    conv_bgroup(0, w0, BGROUPS[3])

    # image 1: tail = 7-row block then the single row, with the last two
    # DMAs on the two fastest fixed-latency queues
    conv_bgroup(1, w1, BGROUPS[0])
    conv_bgroup(1, w1, BGROUPS[1])
    conv_bgroup(1, w1, BLOCKS[6:7])
    conv_bgroup(1, w1, BLOCKS[7:8])
    conv_bgroup(1, w1, BLOCKS[8:9], dma_eng=nc.sync)
    conv_bgroup(1, w1, BLOCKS[9:10], dma_eng=nc.scalar)


def get_nc():
    if "nc" not in _NC_CACHE:
        _NC_CACHE["nc"] = build_nc()
    return _NC_CACHE["nc"]


def _to_bf16(a):
    import ml_dtypes

    return np.ascontiguousarray(a, np.float32).astype(ml_dtypes.bfloat16)


def shard_inputs(x, weight, bias, se_w1, se_w2, se_b2):
    # host-side layout prep of the replicated (batch-independent) params:
    # weight -> [ci, tap, k, co] bf16 (tap-major so each combine tap-group
    # is one contiguous DMA)
    w4 = np.ascontiguousarray(weight, np.float32).reshape(K, CO, CI, 3, 3)
    weight_t = np.ascontiguousarray(
        w4.transpose(2, 3, 4, 0, 1).reshape(CI, 9, K, CO))
    common = dict(
        weight_t=_to_bf16(weight_t),
        bias_cos=np.ascontiguousarray(
            np.asarray(bias, np.float32).reshape(K, CO).T),
        se_w1t=np.ascontiguousarray(np.asarray(se_w1, np.float32).T),
        se_w2b=np.ascontiguousarray(np.concatenate(
            [np.asarray(se_w2, np.float32).T,
             np.asarray(se_b2, np.float32)[None, :]], axis=0)),
    )
    return [
        dict(x2=_to_bf16(x[c * B:(c + 1) * B]), **common)
        for c in range(N_CORES)
    ]


def kernel(x, weight, bias, se_w1, se_w2, se_b2):
    nc = get_nc()
    in_maps = shard_inputs(x, weight, bias, se_w1, se_w2, se_b2)
    res = run_bass_kernel_spmd(nc, in_maps, core_ids=list(range(N_CORES)))
    return np.concatenate(
        [np.asarray(r["y2"]).astype(np.float32) for r in res.results], axis=0)


# revision 20
# speedup vs baseline: 1.7134x; 1.0084x over previous
"""DyConv2d (dynamic convolution with SE attention) on 8 TRN2 NeuronCores.

Reference computation (per image):
    attn = softmax(MLP(global_avg_pool(x)) / T)            # [K=4]
    y    = conv3x3(x, W) + bias                            # W: [K*128, 128, 3, 3]
    out  = sum_k attn[k] * y[k]                            # [128, 64, 64]

Key algebraic rewrite: conv is linear in the weights, so
    out = conv3x3(x, sum_k attn[k] * W_k) + sum_k attn[k] * bias_k
which cuts the conv FLOPs by 4x (one 128->128 conv per image instead of
128->512). A second rewrite removes the softmax normalization from the
critical path: combine with raw exp weights e_k and fold r = 1/sum(e)
into the eviction, out = r * conv(x, sum_k e_k W_k) + r * sum_k e_k b_k
(the eviction's per-partition scale/bias APs apply both for free).

Sharding: data-parallel over batch, 2 images per core. x and the weights
ship as bf16 (the conv consumes bf16 either way; PSUM accumulates fp32),
halving every input DMA; the output returns as bf16 and is upcast on the
host (measured rel err 4.4e-3 vs the 2e-2 gate).

Per-core pipeline (conv start ~8.7us, PE-packed to ~40.4us, tail 3.8us):
  1. x image 0 in 5 chunks (16/16/16/12/4 rows) on the sync queue; DVE
     copies each chunk into the flat-padded conv layout (row pitch 65 so
     every 3x3 tap is one contiguous matmul; pads zeroed by Pool memsets)
     with per-chunk SE sums via accum_out. bf16 weights (tap-major
     [ci,tap,k,co], one DMA per combine tap-group) follow x0, then x1.
     Tiny SE params ride the Pool SWDGE queue; a dummy ACT op at t=0
     absorbs the 1.3us activation-table load off the critical path.
  2. SE on-chip, no DRAM bounces: the hidden layer accumulates per-chunk
     partial matmuls in PSUM (only the last 4-row chunk is on the
     critical path); logits come out transposed as [1,K] by swapping
     matmul operand roles (lhsT = hidden column, rhs = W2^T, bias folded
     in as a 34th row against a constant-1 row of the hidden column);
     ACT exp; broadcast to 128 partitions via a ones-vector matmul.
  3. Weight combine over k on DVE in tap groups (1,2,3,3): per-k bf16
     tensor_scalar then a tensor_add tree (packed 4x/2x DVE modes;
     scalar_tensor_tensor would run 1x).
  4. Conv: block-groups of 7-row PSUM banks (N=454 bf16, 1 cycle/row),
     9 accumulating matmuls per bank, per-block ACT eviction applying
     scale=r and bias=r*sum(e_k b_k), block-group DMAs out on the Pool
     SWDGE queue (keeps HWDGE free for input gen).
  5. Image 1's SE matmuls are spliced between image-0 conv block-groups;
     its tail runs the single-row block before the last 7-row block so
     the final DMA chain (sync queue, no HWDGE contention) starts at the
     last possible eviction with nothing queued ahead of it.
"""

import sys

sys.path.insert(0, "/opt/trn_rl_repo")

import numpy as np

from concourse import bacc, mybir
import concourse.tile as tile
from concourse.bass_utils import run_bass_kernel_spmd

B_TOTAL = 16
N_CORES = 8
B = B_TOTAL // N_CORES  # images per core
CI = 128
CO = 128
K = 4
H = W = 64
TEMP = 30.0
F32 = mybir.dt.float32
BF16 = mybir.dt.bfloat16

# raster order: tap i = (i//3, i%3)
TAPS = [(i // 3, i % 3) for i in range(9)]
XPL = 65 * 66 + 4  # padded-x flat length (extra zeros absorb window overrun)
BLOCKS = [(h0, 7) for h0 in range(0, 63, 7)] + [(63, 1)]
BGROUPS = [BLOCKS[0:3], BLOCKS[3:6], BLOCKS[6:8], BLOCKS[8:10]]
# tap groups for the weight combine: sizes 1,2,3,3 so group 0 is ready asap
TAPG = [(0, 1), (1, 2), (3, 3), (6, 3)]
TAPG_OF = [0, 1, 1, 2, 2, 2, 3, 3, 3]

_NC_CACHE = {}


def build_nc():
    nc = bacc.Bacc("TRN2", target_bir_lowering=False)

    x_d = nc.dram_tensor("x2", [B, CI, H, W], BF16, kind="ExternalInput")
    wt_d = nc.dram_tensor("weight_t", [CI, 9, K, CO], BF16, kind="ExternalInput")
    bc_d = nc.dram_tensor("bias_cos", [CO, K], F32, kind="ExternalInput")
    w1t_d = nc.dram_tensor("se_w1t", [CI, 33], F32, kind="ExternalInput")
    w2b_d = nc.dram_tensor("se_w2b", [34, K], F32, kind="ExternalInput")
    y_d = nc.dram_tensor("y2", [B, CO, H, W], BF16, kind="ExternalOutput")

    with tile.TileContext(nc) as tc:
        with (
            tc.tile_pool(name="consts", bufs=1) as consts,
            tc.tile_pool(name="ximg", bufs=2) as ximg,
            tc.tile_pool(name="weff", bufs=8) as weffp,
            tc.tile_pool(name="cmb", bufs=6) as cmbp,
            tc.tile_pool(name="sesb", bufs=2) as sesb,
            tc.tile_pool(name="ev", bufs=4) as evp,
            tc.tile_pool(name="cv", bufs=6, space="PSUM") as cvp,
            tc.tile_pool(name="tp", bufs=2, space="PSUM") as tpp,
        ):
            build_body(nc, tc, consts, ximg, weffp, cmbp, sesb, evp, cvp, tpp,
                       x_d, wt_d, bc_d, w1t_d, w2b_d, y_d)

    nc.compile()
    return nc


def build_body(nc, tc, consts, ximg, weffp, cmbp, sesb, evp, cvp, tpp,
               x_d, wt_d, bc_d, w1t_d, w2b_d, y_d):
    pool_parts = consts.tile([128, B, 8], F32, tag="pool_parts")
    x_sb = [None, None]
    x_r = [None, None]

    # ---- tiny SE params on the Pool SWDGE queue (keeps HWDGE free) -------
    w1t_sb = consts.tile([CI, 33], F32, tag="w1t_sb")
    nc.gpsimd.dma_start(out=w1t_sb, in_=w1t_d[:, :])
    w2b_sb = consts.tile([34, K], F32, tag="w2b_sb")
    nc.gpsimd.dma_start(out=w2b_sb, in_=w2b_d[:, :])
    bias_cos = consts.tile([CO, K], F32, tag="bias_cos")
    nc.gpsimd.dma_start(out=bias_cos, in_=bc_d[:, :])

    # x chunk row splits: image 0 ends with small chunks so the last
    # bf16-round (the pooled critical path) is short
    XCHUNKS0 = [16, 16, 16, 12, 4]
    XCHUNKS1 = [32, 32]

    def load_x(b, chunks):
        t = ximg.tile([128, H, W], BF16, tag=f"x_sb{b}", name=f"x_sb{b}")
        r0 = 0
        for rows in chunks:
            nc.sync.dma_start(out=t[:, r0:r0 + rows, :],
                              in_=x_d[b, :, r0:r0 + rows, :])
            r0 += rows
        x_sb[b] = t

    load_x(0, XCHUNKS0)

    # raw bf16 weights, tap-major [ci, tap, k, co]; one DMA per tap group,
    # on the same sync queue as x so the ordering x0 -> w -> x1 is exact
    wt_all = consts.tile([128, 9, K, CO], BF16, tag="wt_all")
    for t0, nt in TAPG:
        nc.sync.dma_start(out=wt_all[:, t0:t0 + nt, :, :],
                          in_=wt_d[:, t0:t0 + nt, :, :])

    # constants for the on-chip SE chain (Pool engine memsets, no deps)
    ones_sb = consts.tile([1, 128], F32, tag="ones_sb")
    nc.gpsimd.memset(ones_sb, 1.0)
    # dummy no-dep activation so the ACT function-table load (1.3us) runs
    # at t=0 instead of blocking the first relu on the critical path
    warm_sb = consts.tile([1, 1], F32, tag="warm_sb")
    nc.scalar.activation(out=warm_sb, in_=ones_sb[:, 0:1],
                         func=mybir.ActivationFunctionType.Exp)
    h34 = consts.tile([34, 1], F32, tag="h34")
    nc.gpsimd.memset(h34, 1.0)  # row 33 stays 1.0; relu overwrites rows 0-32

    def pad_xr(b):
        """Allocate the flat-padded bf16 image tile and zero its pads."""
        xr = ximg.tile([128, XPL], BF16, tag=f"x_r{b}", name=f"x_r{b}")
        xr_rows = xr[:, 0:65 * 66].rearrange("p (r c) -> p r c", c=65)
        nc.gpsimd.memset(xr[:, 0:66], 0.0)            # top pad row (+row1 col0)
        nc.gpsimd.memset(xr_rows[:, 2:65, 0], 0.0)    # left pads
        nc.gpsimd.memset(xr[:, 65 * 65:XPL], 0.0)     # bottom pad row + extra
        x_r[b] = xr

    def round_image(b, chunks):
        """copy bf16 x into the flat-padded layout (4x-packed DVE mode);
        per-chunk SE sums go to pool_parts via accum_out (summed by
        accumulating matmuls in se_mm1)."""
        xr = x_r[b]
        xr_rows = xr[:, 0:65 * 66].rearrange("p (r c) -> p r c", c=65)
        r0 = 0
        for q, rows in enumerate(chunks):
            nc.vector.tensor_scalar(
                out=xr_rows[:, 1 + r0:1 + r0 + rows, 1:65],
                in0=x_sb[b][:, r0:r0 + rows, :],
                scalar1=1.0, scalar2=0.0,
                op0=mybir.AluOpType.mult, op1=mybir.AluOpType.add,
                accum_out=pool_parts[:, b, q:q + 1],
            )
            r0 += rows

    pad_xr(0)
    pad_xr(1)
    round_image(0, XCHUNKS0)

    cb_all = consts.tile([128, B], F32, tag="cb_all")
    r_all = consts.tile([128, B], F32, tag="r_all")

    # --- SE chain, split into PE-step functions so image 1's matmuls can
    # be spliced between image-0 conv block-groups without stalling the PE.
    def se_mm1(b, nchunks):
        # h_pre = W1 @ pooled, accumulated per x-chunk in PSUM: chunk q's
        # partial matmul runs as soon as its round lands, so only the last
        # (smallest) chunk is on the critical path
        ps_h = tpp.tile([128, 512], F32, tag="tp", name=f"ps_h{b}")[0:33, 0:1]
        for q in range(nchunks):
            nc.tensor.matmul(ps_h, w1t_sb, pool_parts[:, b, q:q + 1],
                             start=(q == 0), stop=(q == nchunks - 1))
        nc.scalar.activation(out=h34[0:33, :], in_=ps_h,
                             func=mybir.ActivationFunctionType.Relu,
                             scale=1.0 / (H * W))

    def se_mm2(b):
        # logits directly in [1, K] row layout: lhsT = h column, rhs = W2^T
        # (bias folded in via h34[33] = 1, w2b row 33 = b2); softmax done on
        # the tiny row so the broadcast ships already-normalized attention
        ps_lgt = tpp.tile([128, 512], F32, tag="tp", name=f"ps_lgt{b}")[0:1, 0:K]
        nc.tensor.matmul(ps_lgt, h34, w2b_sb, start=True, stop=True)
        e_row = sesb.tile([1, K], F32, tag="e_row", name=f"e_row{b}")
        nc.scalar.activation(out=e_row, in_=ps_lgt,
                             func=mybir.ActivationFunctionType.Exp,
                             scale=1.0 / TEMP)
        return e_row

    def se_bcast(b, e_row):
        # broadcast the UNnormalized exp row to 128 partitions via a
        # ones-vector matmul, then one ACT copy into SBUF (PSUM-sourced
        # scalars cost every DVE combine op an extra ~130ns of init);
        # the softmax 1/sum is folded into the eviction's scale AP
        ps_e = tpp.tile([128, 512], F32, tag="tp", name=f"ps_e{b}")[:, 0:K]
        nc.tensor.matmul(ps_e, ones_sb, e_row, start=True, stop=True)
        e_bc = sesb.tile([128, K], F32, tag="e_bc", name=f"e_bc{b}")
        nc.vector.tensor_scalar(out=e_bc, in0=ps_e, scalar1=1.0, scalar2=None,
                                op0=mybir.AluOpType.mult)
        return e_bc

    def se_norm(b, ps_e):
        # r = 1/sum_k e_k, per partition (off the conv critical path)
        s_bc = sesb.tile([128, 1], F32, tag="s_bc")
        nc.vector.reduce_sum(out=s_bc, in_=ps_e, axis=mybir.AxisListType.X)
        nc.vector.reciprocal(out=r_all[:, b:b + 1], in_=s_bc)

    def comb_bias(b, ps_e):
        # cbr = r * sum_k e_k * bias[k]  (the normalized combined bias)
        tmp = sesb.tile([128, K], F32, tag="cbtmp")
        nc.vector.tensor_mul(tmp, bias_cos, ps_e)
        cbe = sesb.tile([128, 1], F32, tag="cbe")
        nc.vector.reduce_sum(out=cbe, in_=tmp, axis=mybir.AxisListType.X)
        nc.vector.tensor_mul(cb_all[:, b:b + 1], cbe, r_all[:, b:b + 1])

    def combine_group(b, attn_bc, g):
        """weff_g [128, nt, CO] (bf16) = sum_k attn[k] * wt[:, taps_g, k, :].

        All-bf16 tensor_scalar / tensor_tensor run in the DVE's packed 4x
        mode; scalar_tensor_tensor does not, so the sum is a scale/add tree
        rather than a multiply-accumulate chain."""
        t0, nt = TAPG[g]
        sl = slice(t0, t0 + nt)
        tk = []
        for k in range(K):
            t = cmbp.tile([128, 3, CO], BF16, tag="cmb_t",
                          name=f"cmb_t{k}")[:, 0:nt, :]
            nc.vector.tensor_scalar(
                out=t, in0=wt_all[:, sl, k, :], scalar1=attn_bc[:, k:k + 1],
                scalar2=None, op0=mybir.AluOpType.mult)
            tk.append(t)
        t01 = cmbp.tile([128, 3, CO], BF16, tag="cmb_t",
                        name="cmb_t01")[:, 0:nt, :]
        nc.vector.tensor_add(t01, tk[0], tk[1])
        t23 = cmbp.tile([128, 3, CO], BF16, tag="cmb_t",
                        name="cmb_t23")[:, 0:nt, :]
        nc.vector.tensor_add(t23, tk[2], tk[3])
        wg = weffp.tile([128, nt, CO], BF16, tag=f"weff{b}g{g}",
                        name=f"weff{b}g{g}")
        nc.vector.tensor_add(wg, t01, t23)
        return wg

    def conv_bgroup(b, weff, grp, dma_eng=None):
        """One block-group: 9 accumulating taps into len(grp) PSUM banks,
        per-block ACT eviction (bias add) into a group staging tile, one
        Pool-queue DMA per group (or per block when split_dma, to shrink
        the final-DMA tail)."""
        xr = x_r[b]
        pss = [cvp.tile([128, 512], F32, tag="cv", name=f"cv{b}_{grp[0][0]}_{j}")
               for j in range(len(grp))]
        grows = sum(nr for _, nr in grp)
        gh0 = grp[0][0]
        out_sb = evp.tile([128, 21, W], BF16, tag="ev", name=f"ev{b}_{gh0}")
        for i, (ky, kx) in enumerate(TAPS):
            g = TAPG_OF[i]
            lhsT = weff[g][:, i - TAPG[g][0], :]
            off = (ky - 1) * 65 + (kx - 1)
            for j, (ps, (h0, nr)) in enumerate(zip(pss, grp)):
                n = nr * 65 - 1 if nr > 1 else 64
                obase = (h0 + 1) * 65 + 1
                nc.tensor.matmul(
                    ps[:, 0:n], lhsT, xr[:, obase + off:obase + off + n],
                    start=(i == 0), stop=(i == 8),
                )
                if i == 8:
                    r0 = h0 - gh0
                    ps_rows = ps[:, 0:nr * 65].rearrange("p (r c) -> p r c",
                                                         c=65)
                    nc.scalar.activation(
                        out=out_sb[:, r0:r0 + nr, :],
                        in_=ps_rows[:, 0:nr, 0:64],
                        func=mybir.ActivationFunctionType.Identity,
                        bias=cb_all[:, b:b + 1],
                        scale=r_all[:, b:b + 1])
        (dma_eng or nc.gpsimd).dma_start(out=y_d[b, :, gh0:gh0 + grows, :],
                                         in_=out_sb[:, 0:grows, :])

    # ---------------- image 0 prologue ----------------
    load_x(1, XCHUNKS1)  # x image 1 queued behind the weights (sync queue)
    se_mm1(0, len(XCHUNKS0))
    e0 = se_mm2(0)
    p0 = se_bcast(0, e0)
    wg00 = combine_group(0, p0, 0)
    se_norm(0, p0)
    w0 = [wg00] + [combine_group(0, p0, g) for g in range(1, 4)]
    comb_bias(0, p0)

    # conv image 0, with image-1 SE matmuls spliced between block-groups
    conv_bgroup(0, w0, BGROUPS[0])
    round_image(1, XCHUNKS1)
    conv_bgroup(0, w0, BGROUPS[1])
    se_mm1(1, len(XCHUNKS1))  # PE: runs right after group B's last matmul
    e1 = se_mm2(1)   # PE: short wait on ACT relu only
    conv_bgroup(0, w0, BGROUPS[2])
    p1 = se_bcast(1, e1)  # PE broadcast after group C
    wg10 = combine_group(1, p1, 0)
    se_norm(1, p1)
    w1 = [wg10] + [combine_group(1, p1, g) for g in range(1, 4)]
    comb_bias(1, p1)
    conv_bgroup(0, w0, BGROUPS[3])

    # image 1: tail = 7-row block then the single row, with the last two
    # DMAs on the two fastest fixed-latency queues
    conv_bgroup(1, w1, BGROUPS[0])
    conv_bgroup(1, w1, BGROUPS[1])
    conv_bgroup(1, w1, BLOCKS[6:7])
    conv_bgroup(1, w1, BLOCKS[7:8])
    conv_bgroup(1, w1, BLOCKS[9:10], dma_eng=nc.scalar)
    conv_bgroup(1, w1, BLOCKS[8:9], dma_eng=nc.sync)


def get_nc():
    if "nc" not in _NC_CACHE:
        _NC_CACHE["nc"] = build_nc()
    return _NC_CACHE["nc"]


def _to_bf16(a):
    import ml_dtypes

    return np.ascontiguousarray(a, np.float32).astype(ml_dtypes.bfloat16)


def shard_inputs(x, weight, bias, se_w1, se_w2, se_b2):
    # host-side layout prep of the replicated (batch-independent) params:
    # weight -> [ci, tap, k, co] bf16 (tap-major so each combine tap-group
    # is one contiguous DMA)
    w4 = np.ascontiguousarray(weight, np.float32).reshape(K, CO, CI, 3, 3)
    weight_t = np.ascontiguousarray(
        w4.transpose(2, 3, 4, 0, 1).reshape(CI, 9, K, CO))
    common = dict(
        weight_t=_to_bf16(weight_t),
        bias_cos=np.ascontiguousarray(
            np.asarray(bias, np.float32).reshape(K, CO).T),
        se_w1t=np.ascontiguousarray(np.asarray(se_w1, np.float32).T),
        se_w2b=np.ascontiguousarray(np.concatenate(
            [np.asarray(se_w2, np.float32).T,
             np.asarray(se_b2, np.float32)[None, :]], axis=0)),
    )
    return [
        dict(x2=_to_bf16(x[c * B:(c + 1) * B]), **common)
        for c in range(N_CORES)
    ]


def kernel(x, weight, bias, se_w1, se_w2, se_b2):
    nc = get_nc()
    in_maps = shard_inputs(x, weight, bias, se_w1, se_w2, se_b2)
    res = run_bass_kernel_spmd(nc, in_maps, core_ids=list(range(N_CORES)))
    return np.concatenate(
        [np.asarray(r["y2"]).astype(np.float32) for r in res.results], axis=0)


# revision 29
# speedup vs baseline: 1.7164x; 1.0018x over previous
"""DyConv2d (dynamic convolution with SE attention) on 8 TRN2 NeuronCores.

Reference computation (per image):
    attn = softmax(MLP(global_avg_pool(x)) / T)            # [K=4]
    y    = conv3x3(x, W) + bias                            # W: [K*128, 128, 3, 3]
    out  = sum_k attn[k] * y[k]                            # [128, 64, 64]

Key algebraic rewrite: conv is linear in the weights, so
    out = conv3x3(x, sum_k attn[k] * W_k) + sum_k attn[k] * bias_k
which cuts the conv FLOPs by 4x (one 128->128 conv per image instead of
128->512). A second rewrite removes the softmax normalization from the
critical path: combine with raw exp weights e_k and fold r = 1/sum(e)
into the eviction, out = r * conv(x, sum_k e_k W_k) + r * sum_k e_k b_k
(the eviction's per-partition scale/bias APs apply both for free).

Sharding: data-parallel over batch, 2 images per core. x and the weights
ship as bf16 (the conv consumes bf16 either way; PSUM accumulates fp32),
halving every input DMA; the output returns as bf16 and is upcast on the
host (measured rel err 4.4e-3 vs the 2e-2 gate).

Per-core pipeline (conv start ~8.7us, PE-packed to ~40.4us, tail 3.8us):
  1. x image 0 in 6 chunks (16/16/16/10/4/2 rows) on the sync queue; DVE
     copies each chunk into the flat-padded conv layout (row pitch 65 so
     every 3x3 tap is one contiguous matmul; pads zeroed by Pool memsets)
     with per-chunk SE sums via accum_out. bf16 weights (tap-major
     [ci,tap,k,co], one DMA per combine tap-group) follow x0, then x1.
     Tiny SE params ride the Pool SWDGE queue; a dummy ACT op at t=0
     absorbs the 1.3us activation-table load off the critical path.
  2. SE on-chip, no DRAM bounces: the hidden layer accumulates per-chunk
     partial matmuls in PSUM (only the last 4-row chunk is on the
     critical path); logits come out transposed as [1,K] by swapping
     matmul operand roles (lhsT = hidden column, rhs = W2^T, bias folded
     in as a 34th row against a constant-1 row of the hidden column);
     ACT exp; broadcast to 128 partitions via a ones-vector matmul.
  3. Weight combine over k on DVE in tap groups (1,2,3,3): per-k bf16
     tensor_scalar then a tensor_add tree (packed 4x/2x DVE modes;
     scalar_tensor_tensor would run 1x).
  4. Conv: block-groups of 7-row PSUM banks (N=454 bf16, 1 cycle/row),
     9 accumulating matmuls per bank, per-block ACT eviction applying
     scale=r and bias=r*sum(e_k b_k), block-group DMAs out on the Pool
     SWDGE queue (keeps HWDGE free for input gen).
  5. Image 1's SE matmuls are spliced between image-0 conv block-groups;
     its tail runs the single-row block before the last 7-row block so
     the final DMA chain (sync queue, no HWDGE contention) starts at the
     last possible eviction with nothing queued ahead of it.
"""

import sys

sys.path.insert(0, "/opt/trn_rl_repo")

import numpy as np

from concourse import bacc, mybir
import concourse.tile as tile
from concourse.bass_utils import run_bass_kernel_spmd

B_TOTAL = 16
N_CORES = 8
B = B_TOTAL // N_CORES  # images per core
CI = 128
CO = 128
K = 4
H = W = 64
TEMP = 30.0
F32 = mybir.dt.float32
BF16 = mybir.dt.bfloat16

# raster order: tap i = (i//3, i%3)
TAPS = [(i // 3, i % 3) for i in range(9)]
XPL = 65 * 66 + 4  # padded-x flat length (extra zeros absorb window overrun)
BLOCKS = [(h0, 7) for h0 in range(0, 63, 7)] + [(63, 1)]
BGROUPS = [BLOCKS[0:3], BLOCKS[3:6], BLOCKS[6:8], BLOCKS[8:10]]
# tap groups for the weight combine: sizes 1,2,3,3 so group 0 is ready asap
TAPG = [(0, 1), (1, 2), (3, 3), (6, 3)]
TAPG_OF = [0, 1, 1, 2, 2, 2, 3, 3, 3]

_NC_CACHE = {}


def build_nc():
    nc = bacc.Bacc("TRN2", target_bir_lowering=False)

    x_d = nc.dram_tensor("x2", [B, CI, H, W], BF16, kind="ExternalInput")
    wt_d = nc.dram_tensor("weight_t", [CI, 9, K, CO], BF16, kind="ExternalInput")
    bc_d = nc.dram_tensor("bias_cos", [CO, K], F32, kind="ExternalInput")
    w1t_d = nc.dram_tensor("se_w1t", [CI, 33], F32, kind="ExternalInput")
    w2b_d = nc.dram_tensor("se_w2b", [34, K], F32, kind="ExternalInput")
    y_d = nc.dram_tensor("y2", [B, CO, H, W], BF16, kind="ExternalOutput")

    with tile.TileContext(nc) as tc:
        with (
            tc.tile_pool(name="consts", bufs=1) as consts,
            tc.tile_pool(name="ximg", bufs=2) as ximg,
            tc.tile_pool(name="weff", bufs=8) as weffp,
            tc.tile_pool(name="cmb", bufs=6) as cmbp,
            tc.tile_pool(name="sesb", bufs=2) as sesb,
            tc.tile_pool(name="ev", bufs=4) as evp,
            tc.tile_pool(name="cv", bufs=6, space="PSUM") as cvp,
            tc.tile_pool(name="tp", bufs=2, space="PSUM") as tpp,
        ):
            build_body(nc, tc, consts, ximg, weffp, cmbp, sesb, evp, cvp, tpp,
                       x_d, wt_d, bc_d, w1t_d, w2b_d, y_d)

    nc.compile()
    return nc


def build_body(nc, tc, consts, ximg, weffp, cmbp, sesb, evp, cvp, tpp,
               x_d, wt_d, bc_d, w1t_d, w2b_d, y_d):
    pool_parts = consts.tile([128, B, 9], F32, tag="pool_parts")
    x_sb = [None, None]
    x_r = [None, None]

    # ---- tiny SE params on the Pool SWDGE queue (keeps HWDGE free) -------
    w1t_sb = consts.tile([CI, 33], F32, tag="w1t_sb")
    nc.gpsimd.dma_start(out=w1t_sb, in_=w1t_d[:, :])
    w2b_sb = consts.tile([34, K], F32, tag="w2b_sb")
    nc.gpsimd.dma_start(out=w2b_sb, in_=w2b_d[:, :])
    bias_cos = consts.tile([CO, K], F32, tag="bias_cos")
    nc.gpsimd.dma_start(out=bias_cos, in_=bc_d[:, :])

    # x chunk row splits: image 0 ends with small chunks so the last
    # bf16-round (the pooled critical path) is short
    XCHUNKS0 = [16, 16, 16, 10, 4, 2]
    XCHUNKS1 = [32, 32]

    def load_x(b, chunks):
        t = ximg.tile([128, H, W], BF16, tag=f"x_sb{b}", name=f"x_sb{b}")
        r0 = 0
        for rows in chunks:
            nc.sync.dma_start(out=t[:, r0:r0 + rows, :],
                              in_=x_d[b, :, r0:r0 + rows, :])
            r0 += rows
        x_sb[b] = t

    load_x(0, XCHUNKS0)

    # raw bf16 weights, tap-major [ci, tap, k, co]; one DMA per tap group,
    # on the same sync queue as x so the ordering x0 -> w -> x1 is exact
    wt_all = consts.tile([128, 9, K, CO], BF16, tag="wt_all")
    for t0, nt in TAPG:
        nc.sync.dma_start(out=wt_all[:, t0:t0 + nt, :, :],
                          in_=wt_d[:, t0:t0 + nt, :, :])

    # constants for the on-chip SE chain (Pool engine memsets, no deps)
    ones_sb = consts.tile([1, 128], F32, tag="ones_sb")
    nc.gpsimd.memset(ones_sb, 1.0)
    # dummy no-dep activation so the ACT function-table load (1.3us) runs
    # at t=0 instead of blocking the first relu on the critical path
    warm_sb = consts.tile([1, 1], F32, tag="warm_sb")
    nc.scalar.activation(out=warm_sb, in_=ones_sb[:, 0:1],
                         func=mybir.ActivationFunctionType.Exp)
    h34 = consts.tile([34, 1], F32, tag="h34")
    nc.gpsimd.memset(h34, 1.0)  # row 33 stays 1.0; relu overwrites rows 0-32

    def pad_xr(b):
        """Allocate the flat-padded bf16 image tile and zero its pads."""
        xr = ximg.tile([128, XPL], BF16, tag=f"x_r{b}", name=f"x_r{b}")
        xr_rows = xr[:, 0:65 * 66].rearrange("p (r c) -> p r c", c=65)
        nc.gpsimd.memset(xr[:, 0:66], 0.0)            # top pad row (+row1 col0)
        nc.gpsimd.memset(xr_rows[:, 2:65, 0], 0.0)    # left pads
        nc.gpsimd.memset(xr[:, 65 * 65:XPL], 0.0)     # bottom pad row + extra
        x_r[b] = xr

    def round_image(b, chunks):
        """copy bf16 x into the flat-padded layout (4x-packed DVE mode);
        per-chunk SE sums go to pool_parts via accum_out (summed by
        accumulating matmuls in se_mm1)."""
        xr = x_r[b]
        xr_rows = xr[:, 0:65 * 66].rearrange("p (r c) -> p r c", c=65)
        r0 = 0
        for q, rows in enumerate(chunks):
            nc.vector.tensor_scalar(
                out=xr_rows[:, 1 + r0:1 + r0 + rows, 1:65],
                in0=x_sb[b][:, r0:r0 + rows, :],
                scalar1=1.0, scalar2=0.0,
                op0=mybir.AluOpType.mult, op1=mybir.AluOpType.add,
                accum_out=pool_parts[:, b, q:q + 1],
            )
            r0 += rows

    pad_xr(0)
    pad_xr(1)
    round_image(0, XCHUNKS0)

    cb_all = consts.tile([128, B], F32, tag="cb_all")
    r_all = consts.tile([128, B], F32, tag="r_all")

    # --- SE chain, split into PE-step functions so image 1's matmuls can
    # be spliced between image-0 conv block-groups without stalling the PE.
    def se_mm1(b, nchunks):
        # h_pre = W1 @ pooled, accumulated per x-chunk in PSUM: chunk q's
        # partial matmul runs as soon as its round lands, so only the last
        # (smallest) chunk is on the critical path
        ps_h = tpp.tile([128, 512], F32, tag="tp", name=f"ps_h{b}")[0:33, 0:1]
        for q in range(nchunks):
            nc.tensor.matmul(ps_h, w1t_sb, pool_parts[:, b, q:q + 1],
                             start=(q == 0), stop=(q == nchunks - 1))
        nc.scalar.activation(out=h34[0:33, :], in_=ps_h,
                             func=mybir.ActivationFunctionType.Relu,
                             scale=1.0 / (H * W))

    def se_mm2(b):
        # logits directly in [1, K] row layout: lhsT = h column, rhs = W2^T
        # (bias folded in via h34[33] = 1, w2b row 33 = b2); softmax done on
        # the tiny row so the broadcast ships already-normalized attention
        ps_lgt = tpp.tile([128, 512], F32, tag="tp", name=f"ps_lgt{b}")[0:1, 0:K]
        nc.tensor.matmul(ps_lgt, h34, w2b_sb, start=True, stop=True)
        e_row = sesb.tile([1, K], F32, tag="e_row", name=f"e_row{b}")
        nc.scalar.activation(out=e_row, in_=ps_lgt,
                             func=mybir.ActivationFunctionType.Exp,
                             scale=1.0 / TEMP)
        return e_row

    def se_bcast(b, e_row):
        # broadcast the UNnormalized exp row to 128 partitions via a
        # ones-vector matmul, then one ACT copy into SBUF (PSUM-sourced
        # scalars cost every DVE combine op an extra ~130ns of init);
        # the softmax 1/sum is folded into the eviction's scale AP
        ps_e = tpp.tile([128, 512], F32, tag="tp", name=f"ps_e{b}")[:, 0:K]
        nc.tensor.matmul(ps_e, ones_sb, e_row, start=True, stop=True)
        e_bc = sesb.tile([128, K], F32, tag="e_bc", name=f"e_bc{b}")
        nc.vector.tensor_scalar(out=e_bc, in0=ps_e, scalar1=1.0, scalar2=None,
                                op0=mybir.AluOpType.mult)
        return e_bc

    def se_norm(b, ps_e):
        # r = 1/sum_k e_k, per partition (off the conv critical path)
        s_bc = sesb.tile([128, 1], F32, tag="s_bc")
        nc.vector.reduce_sum(out=s_bc, in_=ps_e, axis=mybir.AxisListType.X)
        nc.vector.reciprocal(out=r_all[:, b:b + 1], in_=s_bc)

    def comb_bias(b, ps_e):
        # cbr = r * sum_k e_k * bias[k]  (the normalized combined bias)
        tmp = sesb.tile([128, K], F32, tag="cbtmp")
        nc.vector.tensor_mul(tmp, bias_cos, ps_e)
        cbe = sesb.tile([128, 1], F32, tag="cbe")
        nc.vector.reduce_sum(out=cbe, in_=tmp, axis=mybir.AxisListType.X)
        nc.vector.tensor_mul(cb_all[:, b:b + 1], cbe, r_all[:, b:b + 1])

    def combine_group(b, attn_bc, g):
        """weff_g [128, nt, CO] (bf16) = sum_k attn[k] * wt[:, taps_g, k, :].

        All-bf16 tensor_scalar / tensor_tensor run in the DVE's packed 4x
        mode; scalar_tensor_tensor does not, so the sum is a scale/add tree
        rather than a multiply-accumulate chain."""
        t0, nt = TAPG[g]
        sl = slice(t0, t0 + nt)
        tk = []
        for k in range(K):
            t = cmbp.tile([128, 3, CO], BF16, tag="cmb_t",
                          name=f"cmb_t{k}")[:, 0:nt, :]
            nc.vector.tensor_scalar(
                out=t, in0=wt_all[:, sl, k, :], scalar1=attn_bc[:, k:k + 1],
                scalar2=None, op0=mybir.AluOpType.mult)
            tk.append(t)
        t01 = cmbp.tile([128, 3, CO], BF16, tag="cmb_t",
                        name="cmb_t01")[:, 0:nt, :]
        nc.vector.tensor_add(t01, tk[0], tk[1])
        t23 = cmbp.tile([128, 3, CO], BF16, tag="cmb_t",
                        name="cmb_t23")[:, 0:nt, :]
        nc.vector.tensor_add(t23, tk[2], tk[3])
        wg = weffp.tile([128, nt, CO], BF16, tag=f"weff{b}g{g}",
                        name=f"weff{b}g{g}")
        nc.vector.tensor_add(wg, t01, t23)
        return wg

    def conv_bgroup(b, weff, grp, dma_eng=None):
        """One block-group: 9 accumulating taps into len(grp) PSUM banks,
        per-block ACT eviction (bias add) into a group staging tile, one
        Pool-queue DMA per group (or per block when split_dma, to shrink
        the final-DMA tail)."""
        xr = x_r[b]
        pss = [cvp.tile([128, 512], F32, tag="cv", name=f"cv{b}_{grp[0][0]}_{j}")
               for j in range(len(grp))]
        grows = sum(nr for _, nr in grp)
        gh0 = grp[0][0]
        out_sb = evp.tile([128, 21, W], BF16, tag="ev", name=f"ev{b}_{gh0}")
        for i, (ky, kx) in enumerate(TAPS):
            g = TAPG_OF[i]
            lhsT = weff[g][:, i - TAPG[g][0], :]
            off = (ky - 1) * 65 + (kx - 1)
            for j, (ps, (h0, nr)) in enumerate(zip(pss, grp)):
                n = nr * 65 - 1 if nr > 1 else 64
                obase = (h0 + 1) * 65 + 1
                nc.tensor.matmul(
                    ps[:, 0:n], lhsT, xr[:, obase + off:obase + off + n],
                    start=(i == 0), stop=(i == 8),
                )
                if i == 8:
                    r0 = h0 - gh0
                    ps_rows = ps[:, 0:nr * 65].rearrange("p (r c) -> p r c",
                                                         c=65)
                    nc.scalar.activation(
                        out=out_sb[:, r0:r0 + nr, :],
                        in_=ps_rows[:, 0:nr, 0:64],
                        func=mybir.ActivationFunctionType.Identity,
                        bias=cb_all[:, b:b + 1],
                        scale=r_all[:, b:b + 1])
        (dma_eng or nc.gpsimd).dma_start(out=y_d[b, :, gh0:gh0 + grows, :],
                                         in_=out_sb[:, 0:grows, :])

    # ---------------- image 0 prologue ----------------
    load_x(1, XCHUNKS1)  # x image 1 queued behind the weights (sync queue)
    se_mm1(0, len(XCHUNKS0))
    e0 = se_mm2(0)
    p0 = se_bcast(0, e0)
    wg00 = combine_group(0, p0, 0)
    se_norm(0, p0)
    w0 = [wg00] + [combine_group(0, p0, g) for g in range(1, 4)]
    comb_bias(0, p0)

    # conv image 0, with image-1 SE matmuls spliced between block-groups
    conv_bgroup(0, w0, BGROUPS[0])
    round_image(1, XCHUNKS1)
    conv_bgroup(0, w0, BGROUPS[1])
    se_mm1(1, len(XCHUNKS1))  # PE: runs right after group B's last matmul
    e1 = se_mm2(1)   # PE: short wait on ACT relu only
    conv_bgroup(0, w0, BGROUPS[2])
    p1 = se_bcast(1, e1)  # PE broadcast after group C
    wg10 = combine_group(1, p1, 0)
    se_norm(1, p1)
    w1 = [wg10] + [combine_group(1, p1, g) for g in range(1, 4)]
    comb_bias(1, p1)
    conv_bgroup(0, w0, BGROUPS[3])

    # image 1: tail = 7-row block then the single row, with the last two
    # DMAs on the two fastest fixed-latency queues
    conv_bgroup(1, w1, BGROUPS[0])
    conv_bgroup(1, w1, BGROUPS[1])
    conv_bgroup(1, w1, BLOCKS[6:7])
    conv_bgroup(1, w1, BLOCKS[7:8])
    conv_bgroup(1, w1, BLOCKS[9:10], dma_eng=nc.scalar)
    conv_bgroup(1, w1, BLOCKS[8:9], dma_eng=nc.sync)


def get_nc():
    if "nc" not in _NC_CACHE:
        _NC_CACHE["nc"] = build_nc()
    return _NC_CACHE["nc"]


def _to_bf16(a):
    import ml_dtypes

    return np.ascontiguousarray(a, np.float32).astype(ml_dtypes.bfloat16)


def shard_inputs(x, weight, bias, se_w1, se_w2, se_b2):
    # host-side layout prep of the replicated (batch-independent) params:
    # weight -> [ci, tap, k, co] bf16 (tap-major so each combine tap-group
    # is one contiguous DMA)
    w4 = np.ascontiguousarray(weight, np.float32).reshape(K, CO, CI, 3, 3)
    weight_t = np.ascontiguousarray(
        w4.transpose(2, 3, 4, 0, 1).reshape(CI, 9, K, CO))
    common = dict(
        weight_t=_to_bf16(weight_t),
        bias_cos=np.ascontiguousarray(
            np.asarray(bias, np.float32).reshape(K, CO).T),
        se_w1t=np.ascontiguousarray(np.asarray(se_w1, np.float32).T),
        se_w2b=np.ascontiguousarray(np.concatenate(
            [np.asarray(se_w2, np.float32).T,
             np.asarray(se_b2, np.float32)[None, :]], axis=0)),
    )
    return [
        dict(x2=_to_bf16(x[c * B:(c + 1) * B]), **common)
        for c in range(N_CORES)
    ]


def kernel(x, weight, bias, se_w1, se_w2, se_b2):
    nc = get_nc()
    in_maps = shard_inputs(x, weight, bias, se_w1, se_w2, se_b2)
    res = run_bass_kernel_spmd(nc, in_maps, core_ids=list(range(N_CORES)))
    return np.concatenate(
        [np.asarray(r["y2"]).astype(np.float32) for r in res.results], axis=0)
